# revision 1
# baseline (speedup 1.0000x reference)
"""Trainium2 Bass kernel for nn_CNNModel_82222853915196.

Model (per utterance x: (64, 512)):
  multiscale patch features (h in {8,16,32,64}) -> feats (8192,)
  out[t, :] = Wfc @ concat([x[:, t], feats]) + bfc

Factorization: feats is broadcast over t, so
  out = x.T @ Wfc1.T  +  1 * (Wfc2 @ feats + cconst).T
with Wfc1 = Wfc[:, :64], Wfc2 = Wfc[:, 64:], all feature-bias terms folded
into cconst on the host.

Patch features never materialize an im2col tensor: the patch contraction
  f_h[k,p,o] = sum_{i,j} x[k+i, h*p+j] W_h[k,o,i*h+j]
is computed with "masked" stationary weights over the full 64-row contraction
(rows outside [k, k+h) zeroed host-side), so all offsets k fuse into the
matmul M dim and x is read straight from SBUF with strided APs:
one PSUM-accumulated matmul per within-row offset j.

Weights and feature math run in fp16 (same bytes as bf16, 8x the mantissa);
the frames matmul and final outputs stay fp32. Overall rel err ~4e-4.

Sharding: pure data parallel - 32 utterances -> 8 cores x 4. Weights
replicated; no cross-core communication. DMA issue is spread over the two
HWDGE rings (sync, scalar) + SWDGE (gpsimd) to overlap transfers.
"""

import os
import sys
from contextlib import ExitStack

import numpy as np

for _p in ("/opt/trn_rl_repo", "/root/.axon_site/_ro/trn_rl_repo"):
    if os.path.isdir(_p) and _p not in sys.path:
        sys.path.insert(0, _p)

import concourse.bass as bass
import concourse.tile as tile
from concourse import bacc, mybir
from concourse.bass_utils import run_bass_kernel_spmd

NCORES = 8
NUTT = 4                 # utterances per core
T = 512
F = 64
OUT = 400
W = NUTT * T             # 2048, free width of the x tile
FP32 = mybir.dt.float32
FP16 = mybir.dt.float16
NPF16 = np.float16


# ---------------------------------------------------------------------------
# host-side weight preparation
# ---------------------------------------------------------------------------

def _build_devindex():
    """devindex[kt, fp] = reference flat feature index m in [0, 8192)."""
    devindex = np.full((64, 128), -1, dtype=np.int64)
    # h=8: PSUM (q=k*4+o, u*64+p): kt = p//4, fp = (p%4)*32 + q
    for k in range(8):
        for p in range(64):
            for o in range(4):
                devindex[p // 4, (p % 4) * 32 + k * 4 + o] = (k * 64 + p) * 4 + o
    # h=16: (q=k*16+o, u*32+p): kt = 16 + p//2, fp = (p%2)*64 + q
    for k in range(4):
        for p in range(32):
            for o in range(16):
                devindex[16 + p // 2, (p % 2) * 64 + k * 16 + o] = \
                    2048 + (k * 32 + p) * 16 + o
    # h=32: (q=k*64+o, u*16+p): kt = 32 + p, fp = q
    for k in range(2):
        for p in range(16):
            for o in range(64):
                devindex[32 + p, k * 64 + o] = 4096 + (k * 16 + p) * 64 + o
    # h=64: (u*8+p, o): kt = 48 + p*2 + o//128, fp = o%128
    for p in range(8):
        for o in range(256):
            devindex[48 + p * 2 + o // 128, o % 128] = 6144 + p * 256 + o
    assert devindex.min() >= 0
    return devindex


def _masked(Wh, nk, h, no):
    """w[r, j, k*no+o] = Wh[k, o, (r-k)*h+j] for 0 <= r-k < h else 0."""
    w = np.zeros((64, h, nk * no), dtype=np.float32)
    for k in range(nk):
        for i in range(h):
            w[k + i, :, k * no:(k + 1) * no] = Wh[k].reshape(no, h, h)[:, i, :].T
    return w


def host_prep(W8, b8, W16, b16, W32, b32, W64, b64, Wfc, bfc):
    f32 = np.float32
    W8 = np.asarray(W8, f32); W16 = np.asarray(W16, f32)
    W32 = np.asarray(W32, f32); W64 = np.asarray(W64, f32)
    Wfc = np.asarray(Wfc, f32)
    b8 = np.asarray(b8, f32); b16 = np.asarray(b16, f32)
    b32 = np.asarray(b32, f32); b64 = np.asarray(b64, f32)
    bfc = np.asarray(bfc, f32)

    w8j = _masked(W8, 8, 8, 4).reshape(64, 256)
    w16j = _masked(W16, 4, 16, 16).reshape(64, 1024)
    w32j = _masked(W32, 2, 32, 64).reshape(64, 4096)
    # w64w[i, j*256+o] = W64[o, i*64+j]
    w64w = np.ascontiguousarray(
        W64.reshape(256, 64, 64).transpose(1, 2, 0).reshape(64, 64 * 256))

    devindex = _build_devindex()
    Wfc2 = Wfc[:, 64:]
    wfc2t = np.ascontiguousarray(
        Wfc2[:, devindex.reshape(-1)].T.reshape(64, 128, OUT))
    wfc1t4 = np.ascontiguousarray(np.tile(Wfc[:, :64].T, (1, NUTT)))

    fb = np.zeros(8192, dtype=np.float64)
    fb[0:2048] = np.broadcast_to(b8[:, None, :], (8, 64, 4)).reshape(-1)
    fb[2048:4096] = np.broadcast_to(b16[:, None, :], (4, 32, 16)).reshape(-1)
    fb[4096:6144] = np.broadcast_to(b32[:, None, :], (2, 16, 64)).reshape(-1)
    fb[6144:8192] = np.broadcast_to(b64[None, :], (8, 256)).reshape(-1)
    cconst = (Wfc2.astype(np.float64) @ fb + bfc.astype(np.float64)).astype(f32)

    return {
        "w8j": w8j.astype(NPF16), "w16j": w16j.astype(NPF16),
        "w32j": w32j.astype(NPF16), "w64w": np.ascontiguousarray(w64w.astype(NPF16)),
        "wfc2t": wfc2t.astype(NPF16),
        "wfc1t4": wfc1t4,
        "cconst": np.ascontiguousarray(cconst.reshape(1, OUT) if os.environ.get("K_CC32")
                                       else cconst.reshape(1, OUT).astype(NPF16)),
    }


# ---------------------------------------------------------------------------
# device program
# ---------------------------------------------------------------------------

def build_program(repeat=1, trace_sim=False):
    nc = bacc.Bacc("TRN2", target_bir_lowering=False, debug=False)

    dram = dict(
        x4=nc.dram_tensor("x4", [F, W], FP32, kind="ExternalInput"),
        w8j=nc.dram_tensor("w8j", [64, 256], FP16, kind="ExternalInput"),
        w16j=nc.dram_tensor("w16j", [64, 1024], FP16, kind="ExternalInput"),
        w32j=nc.dram_tensor("w32j", [64, 4096], FP16, kind="ExternalInput"),
        w64w=nc.dram_tensor("w64w", [64, 16384], FP16, kind="ExternalInput"),
        wfc2t=nc.dram_tensor("wfc2t", [64, 128, OUT], FP16, kind="ExternalInput"),
        wfc1t4=nc.dram_tensor("wfc1t4", [64, NUTT * OUT], FP32, kind="ExternalInput"),
        cconst=nc.dram_tensor("cconst", [1, OUT], FP32 if os.environ.get("K_CC32") else FP16, kind="ExternalInput"),
        out=nc.dram_tensor("out", [W, OUT], FP32, kind="ExternalOutput"),
        featsflat=nc.dram_tensor("featsflat", [64, 128, NUTT], FP16),
    )

    with tile.TileContext(nc, trace_sim=trace_sim) as tc:
        for rep in range(repeat):
            with ExitStack() as ctx:
                _emit(nc, tc, ctx, dram, rep)

    nc.compile()
    return nc


def _emit(nc, tc, ctx, dram, rep):
    if os.environ.get("K_ALL_SYNC"):
        class _S:
            dma_start = staticmethod(nc.sync.dma_start)
        scalar_dma = sync_dma = gpsimd_dma = nc.sync.dma_start
    else:
        scalar_dma = nc.scalar.dma_start
        gpsimd_dma = nc.gpsimd.dma_start
        sync_dma = nc.sync.dma_start
    const = ctx.enter_context(tc.tile_pool(name=f"const{rep}", bufs=1))
    stg = ctx.enter_context(tc.tile_pool(name=f"stg{rep}", bufs=2))
    wfc2p = ctx.enter_context(tc.tile_pool(name=f"wfc2p{rep}", bufs=2))
    outp = ctx.enter_context(tc.tile_pool(name=f"outp{rep}", bufs=2))
    ps = ctx.enter_context(tc.tile_pool(name=f"ps{rep}", bufs=2, space="PSUM"))
    psc = ctx.enter_context(tc.tile_pool(name=f"psc{rep}", bufs=1, space="PSUM"))
    psf = ctx.enter_context(tc.tile_pool(name=f"psf{rep}", bufs=2, space="PSUM"))

    CH = 8  # wfc2 k-tiles per streamed chunk

    # ---- input loads. Rings: sync = wfc2 stream; scalar = x4/w64w/out;
    # gpsimd (SWDGE) = small weights, scatters/gathers.
    x4 = const.tile([65, W], FP32, tag="x4")
    scalar_dma(x4[0:64, :], dram["x4"].ap())
    nc.vector.memset(x4[64:65, :], 1.0)
    # fp16 copy of x, duplicated into both 64-partition halves (so operands
    # can sit at base partition 0 or 64 to match w64w's j-parity halves)
    x4h = const.tile([64, W], FP16, tag="x4h")
    nc.vector.tensor_copy(x4h[0:64, :], x4[0:64, :])

    w64w = const.tile([64, 16384], FP16, tag="w64w")
    scalar_dma(w64w[:], dram["w64w"].ap())
    w8j = const.tile([64, 256], FP16, tag="w8j")
    gpsimd_dma(w8j[:], dram["w8j"].ap())
    w16j = const.tile([64, 1024], FP16, tag="w16j")
    gpsimd_dma(w16j[:], dram["w16j"].ap())
    w32j = const.tile([64, 4096], FP16, tag="w32j")
    gpsimd_dma(w32j[:], dram["w32j"].ap())
    cconst = const.tile([1, OUT], FP16 if not os.environ.get("K_CC32") else FP32, tag="cconst")
    gpsimd_dma(cconst[:], dram["cconst"].ap())
    ones1 = const.tile([1, NUTT], FP16 if not os.environ.get("K_CC32") else FP32, tag="ones1")
    nc.vector.memset(ones1[:], 1.0)

    rhs65 = const.tile([65, NUTT * OUT], FP32, tag="rhs65")
    scalar_dma(rhs65[0:64, :], dram["wfc1t4"].ap())

    feats = const.tile([128, 64 * NUTT], FP16, tag="feats")
    cps = psc.tile([NUTT, OUT], FP32, tag="cps")
    featsflat = dram["featsflat"]

    def cmms(b):
        """C matmuls for k-tile block b (16 kts = 2 chunks of CH)."""
        for ch in (2 * b, 2 * b + 1):
            chunk = wfc2p.tile([128, CH * OUT], FP16, tag="wfc2chunk")
            sync_dma(
                chunk[:],
                bass.AP(tensor=dram["wfc2t"], offset=ch * CH * 128 * OUT,
                        ap=[[OUT, 128], [128 * OUT, CH], [1, OUT]]))
            for i in range(CH):
                kt = ch * CH + i
                nc.tensor.matmul(cps[:], feats[:, kt * NUTT:(kt + 1) * NUTT],
                                 chunk[:, i * OUT:(i + 1) * OUT],
                                 start=(kt == 0), stop=False)

    def gather(b):
        gpsimd_dma(
            feats[:, b * 16 * NUTT:(b + 1) * 16 * NUTT],
            bass.AP(tensor=featsflat, offset=b * 16 * 128 * NUTT,
                    ap=[[NUTT, 128], [128 * NUTT, 16], [1, NUTT]]))

    # ---- scale h=8: 8 MMs K=64 M=32 N=256 -> PSUM (k*4+o, u*64+p)
    x8 = x4h[0:64, :].rearrange("i (u p j) -> i u p j", u=NUTT, j=8)
    acc = ps.tile([32, NUTT * 64], FP32, tag="featps")
    for j in range(8):
        nc.tensor.matmul(acc[:], w8j[:, j * 32:(j + 1) * 32], x8[:, :, :, j],
                         start=(j == 0), stop=(j == 7))
    st = stg.tile([32, NUTT * 64], FP16, tag="f8st")
    nc.vector.tensor_copy(st[:], acc[:])
    # scatter (q, u*64+p) -> featsflat[p//4, (p%4)*32+q, u]
    gpsimd_dma(
        bass.AP(tensor=featsflat, offset=0,
                ap=[[NUTT, 32], [1, NUTT], [128 * NUTT, 16], [32 * NUTT, 4]]),
        st[:].rearrange("q (u ph pl) -> q u ph pl", u=NUTT, ph=16))
    gather(0)
    cmms(0)

    # ---- scale h=16: 16 MMs K=64 M=64 N=128 -> PSUM (k*16+o, u*32+p)
    x16 = x4h[0:64, :].rearrange("i (u p j) -> i u p j", u=NUTT, j=16)
    acc = ps.tile([64, NUTT * 32], FP32, tag="featps")
    for j in range(16):
        nc.tensor.matmul(acc[:], w16j[:, j * 64:(j + 1) * 64], x16[:, :, :, j],
                         start=(j == 0), stop=(j == 15))
    st = stg.tile([64, NUTT * 32], FP16, tag="f16st")
    nc.vector.tensor_copy(st[:], acc[:])
    # scatter (q, u*32+p) -> featsflat[16+p//2, (p%2)*64+q, u]
    gpsimd_dma(
        bass.AP(tensor=featsflat, offset=16 * 128 * NUTT,
                ap=[[NUTT, 64], [1, NUTT], [128 * NUTT, 16], [64 * NUTT, 2]]),
        st[:].rearrange("q (u ph pl) -> q u ph pl", u=NUTT, ph=16))
    gather(1)
    cmms(1)

    # ---- scale h=32: 32 MMs K=64 M=128 N=64 -> PSUM (k*64+o, u*16+p)
    x32 = x4h[0:64, :].rearrange("i (u p j) -> i u p j", u=NUTT, j=32)
    acc = ps.tile([128, NUTT * 16], FP32, tag="featps")
    for j in range(32):
        nc.tensor.matmul(acc[:], w32j[:, j * 128:(j + 1) * 128], x32[:, :, :, j],
                         start=(j == 0), stop=(j == 31))
    st = stg.tile([128, NUTT * 16], FP16, tag="f32st")
    nc.vector.tensor_copy(st[:], acc[:])
    # scatter (q, u*16+p) -> featsflat[32+p, q, u]
    gpsimd_dma(
        bass.AP(tensor=featsflat, offset=32 * 128 * NUTT,
                ap=[[NUTT, 128], [1, NUTT], [128 * NUTT, 16]]),
        st[:].rearrange("q (u p) -> q u p", u=NUTT))
    gather(2)
    cmms(2)

    # ---- scale h=64: 64 MMs K=64 M=32 N=256 (x stationary, w64 streamed)
    acc = ps.tile([NUTT * 8, 256], FP32, tag="featps")
    x64 = x4h[0:64, :].rearrange("i (u p j) -> i u p j", u=NUTT, j=64)
    for j in range(64):
        nc.tensor.matmul(acc[:], x64[:, :, :, j],
                         w64w[:, j * 256:(j + 1) * 256],
                         start=(j == 0), stop=(j == 63))
    st = stg.tile([NUTT * 8, 256], FP16, tag="f64st")
    nc.vector.tensor_copy(st[:], acc[:])
    # scatter (u*8+p, o) -> featsflat[48+p*2+o//128, o%128, u]
    for u in range(NUTT):
        gpsimd_dma(
            bass.AP(tensor=featsflat, offset=48 * 128 * NUTT + u,
                    ap=[[2 * 128 * NUTT, 8], [128 * NUTT, 2], [NUTT, 128]]),
            st[u * 8:(u + 1) * 8, :].rearrange("p (g o) -> p g o", g=2))
    gather(3)
    cmms(3)

    # ---- finish C: + cconst, stage, write into rhs65 row 64
    nc.tensor.matmul(cps[:], ones1[:], cconst[:], start=False, stop=True)
    csb = stg.tile([NUTT, OUT], FP32, tag="csb")
    nc.vector.tensor_copy(csb[:], cps[:])
    for u in range(NUTT):
        gpsimd_dma(rhs65[64:65, u * OUT:(u + 1) * OUT], csb[u:u + 1, :])

    # ---- frames matmul: out rows = x^T @ Wfc1^T + 1*(C[u]+cconst)
    for u in range(NUTT):
        fsb = outp.tile([128, 4 * OUT], FP32, tag="framesout")
        for tc_i in range(4):
            fps = psf.tile([128, OUT], FP32, tag="framesps")
            nc.tensor.matmul(
                fps[:],
                x4[:, u * T + tc_i * 128: u * T + (tc_i + 1) * 128],
                rhs65[:, u * OUT:(u + 1) * OUT], start=True, stop=True)
            nc.vector.tensor_copy(fsb[:, tc_i * OUT:(tc_i + 1) * OUT], fps[:])
        scalar_dma(
            bass.AP(tensor=dram["out"], offset=u * T * OUT,
                    ap=[[OUT, 128], [128 * OUT, 4], [1, OUT]]),
            fsb[:])


_NC_CACHE = None


def _get_nc():
    global _NC_CACHE
    if _NC_CACHE is None:
        _NC_CACHE = build_program()
    return _NC_CACHE


# ---------------------------------------------------------------------------
# entry point
# ---------------------------------------------------------------------------

def run(inputs, trace=False, **kw):
    nc = _get_nc()
    prep = host_prep(inputs["W8"], inputs["b8"], inputs["W16"], inputs["b16"],
                     inputs["W32"], inputs["b32"], inputs["W64"], inputs["b64"],
                     inputs["Wfc"], inputs["bfc"])
    batch = np.asarray(inputs["batch"], np.float32)
    in_maps = []
    for c in range(NCORES):
        x4 = np.ascontiguousarray(
            batch[NUTT * c:NUTT * (c + 1)].transpose(1, 0, 2).reshape(F, W))
        m = dict(prep)
        m["x4"] = x4
        in_maps.append(m)
    res = run_bass_kernel_spmd(nc, in_maps, core_ids=list(range(NCORES)),
                               trace=trace, **kw)
    out = np.concatenate([r["out"] for r in res.results], axis=0)
    return out, res


def kernel(**inputs):
    out, _ = run(inputs)
    return out



# revision 7
# speedup vs baseline: 1.3608x; 1.3608x over previous
"""Trainium2 Bass kernel for nn_CNNModel_82222853915196.

Model (per utterance x: (64, 512)):
  multiscale patch features (h in {8,16,32,64}) -> feats (8192,)
  out[t, :] = Wfc @ concat([x[:, t], feats]) + bfc

Factorization: feats is broadcast over t, so
  out = x.T @ Wfc1.T  +  broadcast(C),  C = Wfc2 @ feats + cconst
with Wfc1 = Wfc[:, :64], Wfc2 = Wfc[:, 64:], feature biases folded into
cconst on the host.

Key structure (all feature math fp16, fp32 PSUM accumulation):
 * j-pairing: x is host-duplicated into a [128, 2048] tile whose lower 64
   partitions hold x and upper 64 hold x shifted left one column. Masked
   per-offset patch weights for (j, j+1) stack into one K=128 stationary,
   halving the matmul count of every scale.
 * The fc feature weight (Wfc2, 6.55MB fp16) streams as 8 chunks of
   [128, 8*400] with 6400B contiguous lines, feeding one PSUM-accumulated
   K=128 matmul per feature tile (64 total).
 * h=32/h=64 features never round-trip through DRAM: h=32's PSUM layout is
   already partition-compatible with the C matmul; h=64 is fixed up with
   two PE transposes. Only h=8/h=16 use the DRAM scatter/gather.
 * The frames term is computed TRANSPOSED (out_t[o, t]) with Wfc1 as a
   reused stationary: 16 fat matmuls of N=512, staged in SBUF fp16.
   After C is ready, per-(o-tile, utt) tensor_scalar adds (C as a
   per-partition column, spread across DVE/Pool/Act engines) produce
   fp32 out_t tiles, written with 8KB-contiguous DMA lines. The host
   transposes out_t back.

Sharding: pure data parallel - 32 utterances -> 8 cores x 4. Weights
replicated; no cross-core communication (collectives cost ~70-100us in
cross-core skew under this runtime).
"""

import os
import sys
from contextlib import ExitStack

import numpy as np

for _p in ("/opt/trn_rl_repo", "/root/.axon_site/_ro/trn_rl_repo"):
    if os.path.isdir(_p) and _p not in sys.path:
        sys.path.insert(0, _p)

import concourse.bass as bass
import concourse.tile as tile
from concourse import bacc, mybir
from concourse.bass_utils import run_bass_kernel_spmd

NCORES = 8
NUTT = 4                 # utterances per core
T = 512
F = 64
OUT = 400
W = NUTT * T             # 2048, free width of the x tile
FP32 = mybir.dt.float32
FP16 = mybir.dt.float16
NPF16 = np.float16


# ---------------------------------------------------------------------------
# host-side weight preparation
# ---------------------------------------------------------------------------

def _build_devindex():
    """devindex[kt, fp] = reference flat feature index m in [0, 8192)."""
    devindex = np.full((64, 128), -1, dtype=np.int64)
    # h=8: PSUM (q=k*4+o, u*64+p): kt = p//4, fp = (p%4)*32 + q
    for k in range(8):
        for p in range(64):
            for o in range(4):
                devindex[p // 4, (p % 4) * 32 + k * 4 + o] = (k * 64 + p) * 4 + o
    # h=16: (q=k*16+o, u*32+p): kt = 16 + p//2, fp = (p%2)*64 + q
    for k in range(4):
        for p in range(32):
            for o in range(16):
                devindex[16 + p // 2, (p % 2) * 64 + k * 16 + o] = \
                    2048 + (k * 32 + p) * 16 + o
    # h=32: (q=k*64+o, u*16+p): kt = 32 + p, fp = q  (partition-preserving)
    for k in range(2):
        for p in range(16):
            for o in range(64):
                devindex[32 + p, k * 64 + o] = 4096 + (k * 16 + p) * 64 + o
    # h=64 via PE transpose: kt = 48 + g*8 + p (g = o//128), fp = o%128
    for p in range(8):
        for o in range(256):
            devindex[48 + (o // 128) * 8 + p, o % 128] = 6144 + p * 256 + o
    assert devindex.min() >= 0
    return devindex


def _masked_paired(Wh, nk, h, no):
    """w[r or 64+r, m*nk*no + k*no + o] = Wh[k, o, (r-k)*h + (2m or 2m+1)]."""
    w = np.zeros((64, h, nk * no), dtype=np.float32)
    for k in range(nk):
        for i in range(h):
            w[k + i, :, k * no:(k + 1) * no] = Wh[k].reshape(no, h, h)[:, i, :].T
    # w[r, j, q] -> paired [128, (h//2) * nk*no]
    wp = np.zeros((128, (h // 2) * nk * no), dtype=np.float32)
    q = nk * no
    for m in range(h // 2):
        wp[0:64, m * q:(m + 1) * q] = w[:, 2 * m, :]
        wp[64:128, m * q:(m + 1) * q] = w[:, 2 * m + 1, :]
    return wp


def host_prep(W8, b8, W16, b16, W32, b32, W64, b64, Wfc, bfc):
    f32 = np.float32
    W8 = np.asarray(W8, f32); W16 = np.asarray(W16, f32)
    W32 = np.asarray(W32, f32); W64 = np.asarray(W64, f32)
    Wfc = np.asarray(Wfc, f32)
    b8 = np.asarray(b8, f32); b16 = np.asarray(b16, f32)
    b32 = np.asarray(b32, f32); b64 = np.asarray(b64, f32)
    bfc = np.asarray(bfc, f32)

    w8jp = _masked_paired(W8, 8, 8, 4)          # [128, 4*32]
    w16jp = _masked_paired(W16, 4, 16, 16)      # [128, 8*64]
    w32jp = _masked_paired(W32, 2, 32, 64)      # [128, 16*128]

    # w64wp[i, m*256+o] = W64[o, i*64+2m]; row 64+i holds j=2m+1
    w64 = W64.reshape(256, 64, 64)              # [o, i, j]
    w64wp = np.zeros((128, 32 * 256), dtype=f32)
    for m in range(32):
        w64wp[0:64, m * 256:(m + 1) * 256] = w64[:, :, 2 * m].T
        w64wp[64:128, m * 256:(m + 1) * 256] = w64[:, :, 2 * m + 1].T

    devindex = _build_devindex()
    Wfc2 = Wfc[:, 64:]
    perm = Wfc2[:, devindex.reshape(-1)].T      # [8192, 400], kt-major rows
    wfc2f = np.ascontiguousarray(
        perm.reshape(64, 128, OUT).transpose(1, 0, 2).reshape(128, 64 * OUT))

    wfc1t = np.ascontiguousarray(Wfc[:, :64].T)  # [64, 400]

    fb = np.zeros(8192, dtype=np.float64)
    fb[0:2048] = np.broadcast_to(b8[:, None, :], (8, 64, 4)).reshape(-1)
    fb[2048:4096] = np.broadcast_to(b16[:, None, :], (4, 32, 16)).reshape(-1)
    fb[4096:6144] = np.broadcast_to(b32[:, None, :], (2, 16, 64)).reshape(-1)
    fb[6144:8192] = np.broadcast_to(b64[None, :], (8, 256)).reshape(-1)
    cconst = (Wfc2.astype(np.float64) @ fb + bfc.astype(np.float64)).astype(f32)

    return {
        "w8jp": w8jp.astype(NPF16), "w16jp": w16jp.astype(NPF16),
        "w32jp": w32jp.astype(NPF16),
        "w64wp": np.ascontiguousarray(w64wp.astype(NPF16)),
        "wfc2f": wfc2f.astype(NPF16),
        "wfc1t": wfc1t.astype(NPF16),
        "cconst4": np.ascontiguousarray(np.tile(cconst.reshape(1, OUT), (NUTT, 1))),
        "eye32": np.eye(32, dtype=NPF16),
    }


# ---------------------------------------------------------------------------
# device program
# ---------------------------------------------------------------------------

def build_program():
    nc = bacc.Bacc("TRN2", target_bir_lowering=False, debug=False)

    dram = dict(
        x4hp=nc.dram_tensor("x4hp", [128, W], FP16, kind="ExternalInput"),
        w8jp=nc.dram_tensor("w8jp", [128, 128], FP16, kind="ExternalInput"),
        w16jp=nc.dram_tensor("w16jp", [128, 512], FP16, kind="ExternalInput"),
        w32jp=nc.dram_tensor("w32jp", [128, 2048], FP16, kind="ExternalInput"),
        w64wp=nc.dram_tensor("w64wp", [128, 8192], FP16, kind="ExternalInput"),
        wfc2f=nc.dram_tensor("wfc2f", [128, 64 * OUT], FP16, kind="ExternalInput"),
        wfc1t=nc.dram_tensor("wfc1t", [64, OUT], FP16, kind="ExternalInput"),
        cconst4=nc.dram_tensor("cconst4", [NUTT, OUT], FP32, kind="ExternalInput"),
        eye32=nc.dram_tensor("eye32", [32, 32], FP16, kind="ExternalInput"),
        out_t=nc.dram_tensor("out_t", [OUT, W], FP32, kind="ExternalOutput"),
        featsflat=nc.dram_tensor("featsflat", [32, 128, NUTT], FP16),
    )

    with tile.TileContext(nc) as tc:
        with ExitStack() as ctx:
            _emit(nc, tc, ctx, dram)

    nc.compile()
    return nc


def _emit(nc, tc, ctx, dram):
    scalar_dma = nc.scalar.dma_start
    gpsimd_dma = nc.gpsimd.dma_start
    sync_dma = nc.sync.dma_start

    const = ctx.enter_context(tc.tile_pool(name="const", bufs=1))
    stg = ctx.enter_context(tc.tile_pool(name="stg", bufs=2))
    wfc2p = ctx.enter_context(tc.tile_pool(name="wfc2p", bufs=2))
    outp = ctx.enter_context(tc.tile_pool(name="outp", bufs=2))
    ps = ctx.enter_context(tc.tile_pool(name="ps", bufs=2, space="PSUM"))
    psc = ctx.enter_context(tc.tile_pool(name="psc", bufs=1, space="PSUM"))
    psf = ctx.enter_context(tc.tile_pool(name="psf", bufs=2, space="PSUM"))
    pstp = ctx.enter_context(tc.tile_pool(name="pstp", bufs=1, space="PSUM"))

    CH = 8  # wfc2 k-tiles per streamed chunk

    # ---- input loads.  scalar ring: x + w64 + late wfc2 chunks + out;
    # sync ring: early wfc2 chunks + out; gpsimd SWDGE: small weights,
    # scatters/gathers.
    x4hp = const.tile([128, W], FP16, tag="x4hp")
    scalar_dma(x4hp[:], dram["x4hp"].ap())
    w64wp = const.tile([128, 8192], FP16, tag="w64wp")
    scalar_dma(w64wp[:], dram["w64wp"].ap())

    w8jp = const.tile([128, 128], FP16, tag="w8jp")
    gpsimd_dma(w8jp[:], dram["w8jp"].ap())
    w16jp = const.tile([128, 512], FP16, tag="w16jp")
    gpsimd_dma(w16jp[:], dram["w16jp"].ap())
    w32jp = const.tile([128, 2048], FP16, tag="w32jp")
    gpsimd_dma(w32jp[:], dram["w32jp"].ap())
    wfc1t = const.tile([64, OUT], FP16, tag="wfc1t")
    gpsimd_dma(wfc1t[:], dram["wfc1t"].ap())
    cconst4 = const.tile([NUTT, OUT], FP32, tag="cconst4")
    gpsimd_dma(cconst4[:], dram["cconst4"].ap())
    eye32 = const.tile([32, 32], FP16, tag="eye32")
    gpsimd_dma(eye32[:], dram["eye32"].ap())

    feats8_16 = const.tile([128, 128], FP16, tag="feats8_16")
    f32t = const.tile([128, 64], FP16, tag="f32t")
    tp64sb = const.tile([128, 64], FP16, tag="tp64sb")
    ct_sb = const.tile([128, 16], FP32, tag="ct_sb")
    fout = const.tile([128, 16 * 512], FP16, tag="fout")

    cps = psc.tile([NUTT, OUT], FP32, tag="cps")
    featsflat = dram["featsflat"]

    f32t_r = f32t[:, :].rearrange("q (u p) -> q p u", u=NUTT)      # [128,16,4]
    tp64_r = tp64sb[:, :].rearrange("q (g u p) -> q g p u", g=2, u=NUTT)

    def cstat(kt):
        if kt < 16:
            return feats8_16[:, kt * NUTT:(kt + 1) * NUTT]
        if kt < 32:
            return feats8_16[:, 64 + (kt - 16) * NUTT: 64 + (kt - 15) * NUTT]
        if kt < 48:
            return f32t_r[:, kt - 32, :]
        g, p = (kt - 48) // 8, (kt - 48) % 8
        return tp64_r[:, g, p, :]

    def cmms(b):
        """C matmuls for k-tile block b (16 kts = 2 chunks of CH)."""
        for ch in (2 * b, 2 * b + 1):
            chunk = wfc2p.tile([128, CH * OUT], FP16, tag="wfc2chunk")
            ring = sync_dma if ch < 6 else scalar_dma
            ring(chunk[:],
                 bass.AP(tensor=dram["wfc2f"], offset=ch * CH * OUT,
                         ap=[[64 * OUT, 128], [1, CH * OUT]]))
            for i in range(CH):
                kt = ch * CH + i
                nc.tensor.matmul(cps[:], cstat(kt),
                                 chunk[:, i * OUT:(i + 1) * OUT],
                                 start=(kt == 0), stop=(kt == 63))

    def frames(ots):
        """Transposed frames matmuls for the given o-tiles (stationary Wfc1)."""
        for ot in ots:
            for u in range(NUTT):
                fps = psf.tile([100, 512], FP32, tag="framesps")
                nc.tensor.matmul(
                    fps[:], wfc1t[:, ot * 100:(ot + 1) * 100],
                    x4hp[0:64, u * T:(u + 1) * T], start=True, stop=True)
                idx = ot * NUTT + u
                dst = fout[0:100, idx * 512:(idx + 1) * 512]
                if idx % 2:
                    nc.vector.tensor_copy(dst, fps[:])
                else:
                    nc.scalar.activation(dst, fps[:],
                                         mybir.ActivationFunctionType.Copy)

    # ---- scale h=8: 4 paired MMs K=128 M=32 N=256 -> PSUM (k*4+o, u*64+p)
    x8 = x4hp[:, :].rearrange("i (u p j) -> i u p j", u=NUTT, j=8)
    acc = ps.tile([32, NUTT * 64], FP32, tag="featps")
    for m in range(4):
        nc.tensor.matmul(acc[:], w8jp[:, m * 32:(m + 1) * 32], x8[:, :, :, 2 * m],
                         start=(m == 0), stop=(m == 3))
    st = stg.tile([32, NUTT * 64], FP16, tag="f8st")
    nc.vector.tensor_copy(st[:], acc[:])
    # scatter (q, u*64+p) -> featsflat[p//4, (p%4)*32+q, u]
    gpsimd_dma(
        bass.AP(tensor=featsflat, offset=0,
                ap=[[NUTT, 32], [1, NUTT], [128 * NUTT, 16], [32 * NUTT, 4]]),
        st[:].rearrange("q (u ph pl) -> q u ph pl", u=NUTT, ph=16))

    # ---- scale h=16: 8 paired MMs K=128 M=64 N=128 -> PSUM (k*16+o, u*32+p)
    x16 = x4hp[:, :].rearrange("i (u p j) -> i u p j", u=NUTT, j=16)
    acc = ps.tile([64, NUTT * 32], FP32, tag="featps")
    for m in range(8):
        nc.tensor.matmul(acc[:], w16jp[:, m * 64:(m + 1) * 64], x16[:, :, :, 2 * m],
                         start=(m == 0), stop=(m == 7))
    st = stg.tile([64, NUTT * 32], FP16, tag="f16st")
    nc.vector.tensor_copy(st[:], acc[:])
    # scatter (q, u*32+p) -> featsflat[16+p//2, (p%2)*64+q, u]
    gpsimd_dma(
        bass.AP(tensor=featsflat, offset=16 * 128 * NUTT,
                ap=[[NUTT, 64], [1, NUTT], [128 * NUTT, 16], [64 * NUTT, 2]]),
        st[:].rearrange("q (u ph pl) -> q u ph pl", u=NUTT, ph=16))

    # gathers for kt blocks 0-1 (h=8, h=16)
    for b in range(2):
        gpsimd_dma(
            feats8_16[:, b * 16 * NUTT:(b + 1) * 16 * NUTT],
            bass.AP(tensor=featsflat, offset=b * 16 * 128 * NUTT,
                    ap=[[NUTT, 128], [128 * NUTT, 16], [1, NUTT]]))

    frames((0, 1))
    cmms(0)
    frames((2, 3))

    # ---- scale h=32: 16 paired MMs K=128 M=128 N=64 -> PSUM (k*64+o, u*16+p)
    x32 = x4hp[:, :].rearrange("i (u p j) -> i u p j", u=NUTT, j=32)
    acc = ps.tile([128, NUTT * 16], FP32, tag="featps")
    for m in range(16):
        nc.tensor.matmul(acc[:], w32jp[:, m * 128:(m + 1) * 128],
                         x32[:, :, :, 2 * m], start=(m == 0), stop=(m == 15))
    nc.vector.tensor_copy(f32t[:], acc[:])  # direct: fp = q, no DRAM trip

    cmms(1)

    # ---- scale h=64: 32 paired MMs K=128 M=32 N=256 (x stationary, w moving)
    acc = ps.tile([NUTT * 8, 256], FP32, tag="featps")
    x64 = x4hp[:, :].rearrange("i (u p j) -> i u p j", u=NUTT, j=64)
    for m in range(32):
        nc.tensor.matmul(acc[:], x64[:, :, :, 2 * m],
                         w64wp[:, m * 256:(m + 1) * 256],
                         start=(m == 0), stop=(m == 31))
    st64 = stg.tile([NUTT * 8, 256], FP16, tag="f64st")
    nc.vector.tensor_copy(st64[:], acc[:])
    # PE-transpose [32, 128]x2 -> [128, 32]: tp64sb[o%128, g*32 + u*8 + p]
    for g in range(2):
        tpp = pstp.tile([128, 32], FP16, tag="tp64ps")
        nc.tensor.transpose(tpp[:], st64[:, g * 128:(g + 1) * 128], eye32[:])
        nc.vector.tensor_copy(tp64sb[:, g * 32:(g + 1) * 32], tpp[:])

    cmms(2)
    cmms(3)

    # ---- C row: cps + cconst4, then PE-transpose to ct_sb[o%100, ot*4+u]
    csb16 = stg.tile([NUTT, OUT], FP16, tag="csb16")
    nc.vector.tensor_tensor(csb16[:], cps[:], cconst4[:], mybir.AluOpType.add)
    ctp = pstp.tile([128, 16], FP16, tag="ctps")
    for ot in range(4):
        nc.tensor.transpose(ctp[0:100, ot * 4:(ot + 1) * 4],
                            csb16[:, ot * 100:(ot + 1) * 100], eye32[0:4, 0:4])
    nc.vector.tensor_copy(ct_sb[0:100, :], ctp[0:100, :])

    # ---- finalize: out_t[ot*100 + p, t] = fout + C column, fat DMA out
    engines = (nc.vector, nc.gpsimd, nc.scalar)
    for ot in range(4):
        outstage = outp.tile([128, W], FP32, tag="outstage")
        for u in range(NUTT):
            idx = ot * NUTT + u
            eng = engines[idx % 3]
            if eng is nc.scalar:
                eng.activation(outstage[0:100, u * 512:(u + 1) * 512],
                               fout[0:100, idx * 512:(idx + 1) * 512],
                               mybir.ActivationFunctionType.Identity,
                               bias=ct_sb[0:100, idx:idx + 1], scale=1.0)
            else:
                eng.tensor_scalar_add(outstage[0:100, u * 512:(u + 1) * 512],
                                      fout[0:100, idx * 512:(idx + 1) * 512],
                                      ct_sb[0:100, idx:idx + 1])
        ring = sync_dma if ot < 2 else scalar_dma
        ring(bass.AP(tensor=dram["out_t"], offset=ot * 100 * W,
                     ap=[[W, 100], [1, W]]),
             outstage[0:100, :])


_NC_CACHE = None


def _get_nc():
    global _NC_CACHE
    if _NC_CACHE is None:
        _NC_CACHE = build_program()
    return _NC_CACHE


# ---------------------------------------------------------------------------
# entry point
# ---------------------------------------------------------------------------

def run(inputs, trace=False, **kw):
    nc = _get_nc()
    prep = host_prep(inputs["W8"], inputs["b8"], inputs["W16"], inputs["b16"],
                     inputs["W32"], inputs["b32"], inputs["W64"], inputs["b64"],
                     inputs["Wfc"], inputs["bfc"])
    batch = np.asarray(inputs["batch"], np.float32)
    in_maps = []
    for c in range(NCORES):
        x4 = batch[NUTT * c:NUTT * (c + 1)].transpose(1, 0, 2).reshape(F, W)
        x4hp = np.zeros((128, W), dtype=NPF16)
        x4hp[0:64, :] = x4.astype(NPF16)
        x4hp[64:128, 0:W - 1] = x4[:, 1:].astype(NPF16)
        m = dict(prep)
        m["x4hp"] = x4hp
        in_maps.append(m)
    res = run_bass_kernel_spmd(nc, in_maps, core_ids=list(range(NCORES)),
                               trace=trace, **kw)
    out = np.concatenate(
        [np.ascontiguousarray(r["out_t"].T) for r in res.results], axis=0)
    return out, res


def kernel(**inputs):
    out, _ = run(inputs)
    return out
